# revision 2
# baseline (speedup 1.0000x reference)
"""Trainium2 Bass kernel for batched multi-head self-attention (v2).

Problem: x [8, 1500, 768], 12 heads x 64 dims, torch-Linear style projections.
Strategy: data-parallel over batch (1 element per NeuronCore, 8 cores).

v2 changes vs v1 (both validated against the same reference):
  - exp reads score PSUM directly (no fp16 SBUF staging): kills ~27M
    elements of copy traffic that v1 split between DVE and ScalarE.
  - software pipelining at unit granularity (unit = (head-pair, q-block)):
    the ctx matmuls + normalization of unit u-1, the K/Q projections of
    pair p+1, and the finished q-block's output projection all run as PE
    "fillers" inside unit u's score stream, so no engine ever sits behind
    a serial phase boundary.
  - normalization: reciprocal of the denominators row -> PE ones-matmul
    partition-broadcast -> multiply, all given >=2 emission slots of
    slack so the PE never blocks on the DVE chain.
  - PSUM: tag "sc" = 3 x [128,1024] f32 (scores ring; kq/V/out tiles
    sublease the same slots), tag "ctx" = 1 x [65,1024] f32 (ctx psum,
    also hosts the broadcast tile) -> exactly 8 banks.

Matmul operands are bf16 (full PE rate; ~0.4% rel error, inside the 2e-2
gate); PSUM accumulation fp32.
"""

import numpy as np
from contextlib import ExitStack

import ml_dtypes

import concourse.bass as bass
import concourse.bacc as bacc
import concourse.tile as tile
from concourse import mybir
from concourse import bass_utils

F32 = mybir.dt.float32
F32R = mybir.dt.float32r
BF16 = mybir.dt.bfloat16
AF = mybir.ActivationFunctionType
OP = mybir.AluOpType

P = 128
D = 768
H = 12
DH = 64
NE = D // P          # 6 e-chunks (head pairs)
ND = D // P          # 6 d-chunks
SCALE = 0.125
S_FULL = 1500
QB = 512
EH = 384             # half of D for the V projection moving dim


def _chunks(total, size):
    out = []
    o = 0
    while o < total:
        out.append((o, min(size, total - o)))
        o += size
    return out


def build_attention(tc, ctx, xT, wqT, wkT, wvT, woT, bqs, out, S, reps=1):
    """Emit the single-core attention program.

    xT:  [D, S] bf16 DRAM     (x^T for this batch element)
    wqT/wkT/wvT/woT: [D, D] bf16 DRAM  (W.T of the torch-Linear weights)
    bqs: [P, NE] f32 DRAM     (0.125*bq laid out [partition, e-chunk])
    out: [S, D] f32 DRAM      (missing the constant bv@Wo.T+bo row)
    """
    nc = tc.nc
    SC = _chunks(S, P)            # k-chunks, e.g. 11x128 + 92
    QBS = _chunks(S, QB)          # q-blocks 512/512/476
    NSC = len(SC)
    NQ = len(QBS)

    const = ctx.enter_context(tc.tile_pool(name="const", bufs=1))
    qkv = ctx.enter_context(tc.tile_pool(name="qkv", bufs=1))
    ps = ctx.enter_context(tc.tile_pool(name="ps", bufs=3, space="PSUM"))
    cps_pool = ctx.enter_context(tc.tile_pool(name="cpsp", bufs=1, space="PSUM"))
    e_pool = ctx.enter_context(tc.tile_pool(name="epool", bufs=2))
    nrm_pool = ctx.enter_context(tc.tile_pool(name="nrm", bufs=2))
    ctxn_pool = ctx.enter_context(tc.tile_pool(name="ctxn", bufs=3))
    out_sb_pool = ctx.enter_context(tc.tile_pool(name="outsb", bufs=3))
    kt_pool = ctx.enter_context(tc.tile_pool(name="ktp", bufs=2))
    x_pool = ctx.enter_context(tc.tile_pool(name="xp", bufs=1))

    # ---- persistent operands, loaded ONCE (outside the rep loop) ----
    bq_sb = const.tile([P, NE], F32)
    nc.sync.dma_start(out=bq_sb[:], in_=bqs)
    w_sbs = {}
    for key, wdram in (("q", wqT), ("k", wkT), ("v", wvT), ("o", woT)):
        w_sb = const.tile([P, ND, D], BF16, name=f"w_{key}")
        for dc in range(ND):
            nc.sync.dma_start(out=w_sb[:, dc, :],
                              in_=wdram[dc * P:(dc + 1) * P, :])
        w_sbs[key] = w_sb
    ones_sb = const.tile([1, DH], F32)
    nc.vector.memset(ones_sb[:], 1.0)

    V = qkv.tile([P, NSC, H * (DH + 1)], BF16)   # per-head 65th ones column
    # Fill all of V with 1.0 once: the projection evictions overwrite the
    # 64 data columns per head, leaving column DH as the all-ones column
    # that accumulates softmax denominators in the ctx matmul. bf16 memset
    # isn't a valid ISA op, so memset the f32-bitcast view with the bit
    # pattern of two packed bf16 1.0s (0x3F803F80).
    two_bf16_ones = float(np.frombuffer(
        np.uint32(0x3F803F80).tobytes(), dtype=np.float32)[0])
    nc.vector.memset(V[:, :, :].bitcast(F32), two_bf16_ones)

    def body():
        _emit_body(tc, nc, xT, out, S, SC, QBS, NSC, NQ,
                   V, bq_sb, w_sbs, ones_sb, ps, cps_pool, e_pool,
                   nrm_pool, ctxn_pool, out_sb_pool, kt_pool, x_pool)

    if reps == 1:
        body()
    else:
        with tc.For_i(0, reps, 1):
            body()


def _emit_body(tc, nc, xT, out, S, SC, QBS, NSC, NQ,
               V, bq_sb, w_sbs, ones_sb, ps, cps_pool, e_pool,
               nrm_pool, ctxn_pool, out_sb_pool, kt_pool, x_pool):

    # x^T for this rep: 6 flat 2D DMAs (contiguous source rows lower to
    # single hardware descriptors; a fancy 3D pattern costs ~30x more).
    xT_sb = x_pool.tile([P, ND, S], BF16, tag="xt", name="xT_sb")
    for dc in range(ND):
        nc.sync.dma_start(out=xT_sb[:, dc, :],
                          in_=xT[dc * P:(dc + 1) * P, :])

    # per-q-block normalized ctx [e, s], persistent across pairs in a rep
    cns = [ctxn_pool.tile([P, NE, QB], BF16, tag="cn", name=f"cn{_q}")
           for _q in range(NQ)]

    # ---------------- per-piece emitters ----------------

    def emit_kq_tile(pr, kind, qi):
        """One K or Q projection psum tile (all 6 dc accumulations) plus
        its DVE eviction — complete within one emission slot."""
        (q0, qw) = QBS[qi]
        w_sb = w_sbs[kind]
        kq_ps = ps.tile([P, 1024], F32, tag="sc", name="kq_ps")
        for dc in range(ND):
            nc.tensor.matmul(
                kq_ps[:, :qw],
                w_sb[:, dc, pr * P:(pr + 1) * P],
                xT_sb[:, dc, q0:q0 + qw],
                start=(dc == 0), stop=(dc == ND - 1))
        kt_t, qt_t = kt_tiles[pr % 2]
        if kind == "q":
            nc.vector.tensor_scalar(
                out=qt_t[:, q0:q0 + qw], in0=kq_ps[:, :qw],
                scalar1=SCALE, scalar2=bq_sb[:, pr:pr + 1],
                op0=OP.mult, op1=OP.add)
        else:
            nc.vector.tensor_copy(out=kt_t[:, q0:q0 + qw],
                                  in_=kq_ps[:, :qw])

    def emit_v_chunk(kc):
        (s0, sw) = SC[kc]
        for eh in range(D // EH):
            v_ps = ps.tile([P, 1024], F32, tag="sc", name="v_ps")
            for dc in range(ND):
                nc.tensor.matmul(
                    v_ps[:sw, :EH],
                    xT_sb[:, dc, s0:s0 + sw],
                    w_sbs["v"][:, dc, eh * EH:(eh + 1) * EH],
                    start=(dc == 0), stop=(dc == ND - 1))
            vh = V[:sw, kc, :].rearrange("p (h w) -> p h w", w=DH + 1)
            nc.vector.tensor_copy(
                out=vh[:, eh * (EH // DH):(eh + 1) * (EH // DH), 0:DH],
                in_=v_ps[:sw, :EH].rearrange("p (h w) -> p h w", w=DH))

    def emit_scores_kc(u, kc, e_all):
        """QK^T for one k-chunk into a fresh sc psum tile, then exp it
        straight out of PSUM into e_all (full-width: the ragged q-block's
        dead columns exp garbage nobody reads)."""
        (pr, qi) = u
        (q0, qw) = QBS[qi]
        (k0, kw) = SC[kc]
        kt_t, qt_t = kt_tiles[pr % 2]
        sp = ps.tile([P, 1024], F32, tag="sc", name="sp")
        for hi in range(2):
            nc.tensor.matmul(
                sp[:kw, hi * 512:hi * 512 + qw],
                kt_t[hi * DH:(hi + 1) * DH, k0:k0 + kw],
                qt_t[hi * DH:(hi + 1) * DH, q0:q0 + qw],
                start=True, stop=True)
        nc.scalar.activation(out=e_all[:kw, kc, :], in_=sp[:kw, :],
                             func=AF.Exp)

    def emit_ctx_kc(u, kc, e_all, cps):
        (pr, qi) = u
        (q0, qw) = QBS[qi]
        (k0, kw) = SC[kc]
        for hi in range(2):
            h = 2 * pr + hi
            nc.tensor.matmul(
                cps[:, hi * 512:hi * 512 + qw],
                V[:kw, kc, h * (DH + 1):(h + 1) * (DH + 1)],
                e_all[:kw, kc, hi * 512:hi * 512 + qw],
                start=(kc == 0), stop=(kc == NSC - 1))

    def emit_norm_a(u, cps, st):
        """craw copy + reciprocal: first half of normalization; releases
        the ctx psum slot for the next unit."""
        craw = nrm_pool.tile([DH, 1024], F32, tag="craw", name="craw")
        nc.vector.tensor_copy(out=craw[:], in_=cps[0:DH, :])
        rc = nrm_pool.tile([1, 1024], F32R, tag="rc", name="rc")
        with nc.allow_low_precision(reason="f32r reciprocal, fp32 bits"):
            nc.vector.reciprocal(out=rc[:], in_=cps[DH:DH + 1, :])
        st["craw"], st["rc"] = craw, rc

    def emit_norm_b(u, st):
        """ones-matmul partition-broadcast of the reciprocal row."""
        rb_ps = cps_pool.tile([DH, 1024], F32, tag="ctx", name="rb_ps")
        for hi in range(2):  # matmul output must stay within one psum bank
            co = hi * 512
            nc.tensor.matmul(rb_ps[:, co:co + 512],
                             ones_sb[:, :].bitcast(F32R),
                             st["rc"][:, co:co + 512], start=True, stop=True)
        st["rb"] = rb_ps

    def emit_norm_c(u, st):
        (pr, qi) = u
        (q0, qw) = QBS[qi]
        for hi in range(2):
            co = hi * 512
            nc.vector.tensor_tensor(
                out=cns[qi][hi * DH:(hi + 1) * DH, pr, 0:qw],
                in0=st["craw"][:, co:co + qw], in1=st["rb"][:, co:co + qw],
                op=OP.mult)

    out_tiles = {}

    def emit_out_tile(qi, sc_i, half):
        """One output-projection psum tile (6 ec accumulations) + eviction
        (+ DMA store when the second half of the s-chunk completes)."""
        (q0, qw) = QBS[qi]
        (s0, sw) = _chunks(qw, P)[sc_i]
        (o0, ow) = ((0, 512), (512, 256))[half]
        op_t = ps.tile([P, 1024], F32, tag="sc", name="op_t")
        for ec in range(NE):
            nc.tensor.matmul(
                op_t[:sw, :ow],
                cns[qi][:, ec, s0:s0 + sw],
                w_sbs["o"][:, ec, o0:o0 + ow],
                start=(ec == 0), stop=(ec == NE - 1))
        if half == 0:
            out_tiles[(qi, sc_i)] = out_sb_pool.tile(
                [P, D], F32, tag="ot", name=f"ot{qi}_{sc_i}")
        ot = out_tiles[(qi, sc_i)]
        nc.vector.tensor_copy(out=ot[:sw, o0:o0 + ow], in_=op_t[:sw, :ow])
        if half == 1:
            nc.sync.dma_start(out=out[q0 + s0:q0 + s0 + sw, :],
                              in_=ot[:sw, :])

    def phase3_items(qi):
        return [(qi, sc_i, half)
                for sc_i in range(len(_chunks(QBS[qi][1], P)))
                for half in range(2)]

    # ---------------- the software-pipelined schedule ----------------
    #
    # Units run in (pair-major) order; inside unit u's 12-slot score
    # stream we interleave, as PE fillers:
    #   - ctx matmuls of unit u-1 (slots 0-4), its normalization
    #     (craw+recip at 5, broadcast at 8, multiplies at 10),
    #   - K/Q projection tiles of pair p+1 (slots 2/5/8 of qi=1,2 units),
    #   - output-projection tiles of a completed q-block (slots 2..9 of
    #     the following unit),
    #   - the V projection (pair 0, q-block 0 only).
    # Everything left at the end (ctx+norm of the last unit, the last two
    # output projections) drains after the unit loop; under reps it
    # overlaps the next rep's x-load and pair-0 K/Q window.

    kt_tiles = [None, None]

    def alloc_kt(pr):
        kt_tiles[pr % 2] = (
            kt_pool.tile([P, S], BF16, tag="kt", name=f"kt{pr}"),
            kt_pool.tile([P, S], BF16, tag="qt", name=f"qt{pr}"))

    units = [(pr, qi) for pr in range(NE) for qi in range(NQ)]
    # ctx k-chunks of the pending unit, compressed into slots 0-4
    CTX_SLOT_BOUNDS = [0, 3, 6, 8, 10, NSC]

    # pair 0 K/Q window at rep start (nothing to hide it behind within
    # the rep; across reps it overlaps the previous rep's drain)
    alloc_kt(0)
    for qi in range(NQ):
        for kind in ("k", "q"):
            emit_kq_tile(0, kind, qi)

    pending = None       # (unit, e_all, cps, state) awaiting ctx+norm
    phase3_queue = []    # out-projection tiles ready to fill into slots

    for ui, u in enumerate(units):
        (pr, qi) = u
        e_all = e_pool.tile([P, NSC, 1024], BF16, tag="e", name=f"e{ui}")
        if pending is not None:
            pu, pe, pst = pending
            pcps = cps_pool.tile([DH + 1, 1024], F32, tag="ctx", name="cps")
        # kq filler plan for pair p+1: K tiles in the qi==1 unit, Q tiles
        # in the qi==2 unit, at slots 2/5/8
        kq_plan = {}
        if pr + 1 < NE and qi in (1, 2):
            if qi == 1:
                alloc_kt(pr + 1)
            kind = "k" if qi == 1 else "q"
            kq_plan = {2: (kind, 0), 5: (kind, 1), 8: (kind, 2)}

        for kc in range(NSC):
            emit_scores_kc(u, kc, e_all)
            if pending is not None:
                if kc < 5:
                    for pkc in range(CTX_SLOT_BOUNDS[kc],
                                     CTX_SLOT_BOUNDS[kc + 1]):
                        emit_ctx_kc(pu, pkc, pe, pcps)
                elif kc == 5:
                    emit_norm_a(pu, pcps, pst)
                elif kc == 8:
                    emit_norm_b(pu, pst)
                elif kc == 10:
                    emit_norm_c(pu, pst)
                    if pu[0] == NE - 1:
                        phase3_queue.extend(phase3_items(pu[1]))
            if kc in kq_plan:
                kind, j = kq_plan[kc]
                emit_kq_tile(pr + 1, kind, j)
            if u == (0, 0):
                emit_v_chunk(kc)
            if 2 <= kc <= 9 and phase3_queue:
                emit_out_tile(*phase3_queue.pop(0))

        pending = (u, e_all, {})

    # ---------------- drain ----------------
    pu, pe, pst = pending
    pcps = cps_pool.tile([DH + 1, 1024], F32, tag="ctx", name="cps_last")
    for pkc in range(NSC):
        emit_ctx_kc(pu, pkc, pe, pcps)
        if phase3_queue:                      # phase3 of q-block 1
            emit_out_tile(*phase3_queue.pop(0))
    emit_norm_a(pu, pcps, pst)
    emit_norm_b(pu, pst)
    emit_norm_c(pu, pst)
    phase3_queue.extend(phase3_items(pu[1]))
    while phase3_queue:
        emit_out_tile(*phase3_queue.pop(0))


def build_nc(S=S_FULL, reps=1):
    nc = bacc.Bacc("TRN2", target_bir_lowering=False, debug=False,
                   enable_asserts=False, num_devices=1)
    xT = nc.dram_tensor("xT", [D, S], BF16, kind="ExternalInput").ap()
    wqT = nc.dram_tensor("wqT", [D, D], BF16, kind="ExternalInput").ap()
    wkT = nc.dram_tensor("wkT", [D, D], BF16, kind="ExternalInput").ap()
    wvT = nc.dram_tensor("wvT", [D, D], BF16, kind="ExternalInput").ap()
    woT = nc.dram_tensor("woT", [D, D], BF16, kind="ExternalInput").ap()
    bqs = nc.dram_tensor("bqs", [P, NE], F32, kind="ExternalInput").ap()
    out = nc.dram_tensor("out", [S, D], F32, kind="ExternalOutput").ap()
    with tile.TileContext(nc) as tc:
        with ExitStack() as ctx:
            build_attention(tc, ctx, xT, wqT, wkT, wvT, woT, bqs, out, S, reps)
    nc.compile()
    return nc


_NC_CACHE = {}


def _get_nc(S=S_FULL, reps=1):
    if (S, reps) not in _NC_CACHE:
        _NC_CACHE[(S, reps)] = build_nc(S, reps)
    return _NC_CACHE[(S, reps)]


def prep_inputs(x, Wq, bq, Wk, Wv, bv, Wo, bo):
    x = np.asarray(x, dtype=np.float32)
    Wq = np.asarray(Wq, dtype=np.float32)
    Wk = np.asarray(Wk, dtype=np.float32)
    Wv = np.asarray(Wv, dtype=np.float32)
    Wo = np.asarray(Wo, dtype=np.float32)
    bq = np.asarray(bq, dtype=np.float32)
    bv = np.asarray(bv, dtype=np.float32)
    bo = np.asarray(bo, dtype=np.float32)
    bf = ml_dtypes.bfloat16
    xT = np.ascontiguousarray(x.transpose(0, 2, 1)).astype(bf)
    base = {
        "wqT": np.ascontiguousarray(Wq.T).astype(bf),
        "wkT": np.ascontiguousarray(Wk.T).astype(bf),
        "wvT": np.ascontiguousarray(Wv.T).astype(bf),
        "woT": np.ascontiguousarray(Wo.T).astype(bf),
        "bqs": np.ascontiguousarray((SCALE * bq).reshape(NE, P).T),
    }
    const_row = (bv @ Wo.T + bo).astype(np.float32)
    in_maps = [dict(base, xT=np.ascontiguousarray(xT[b])) for b in range(x.shape[0])]
    return in_maps, const_row


def kernel(x, Wq, bq, Wk, Wv, bv, Wo, bo):
    in_maps, const_row = prep_inputs(x, Wq, bq, Wk, Wv, bv, Wo, bo)
    nc = _get_nc(x.shape[1])
    res = bass_utils.run_bass_kernel_spmd(
        nc, in_maps, core_ids=list(range(len(in_maps))))
    out = np.stack([r["out"] for r in res.results])
    return (out + const_row[None, None, :]).astype(np.float32)


# revision 3
# speedup vs baseline: 6510.2258x; 6510.2258x over previous
"""Trainium2 Bass kernel for batched multi-head self-attention (v2).

Problem: x [8, 1500, 768], 12 heads x 64 dims, torch-Linear style projections.
Strategy: data-parallel over batch (1 element per NeuronCore, 8 cores).

v2 changes vs v1 (both validated against the same reference):
  - exp reads score PSUM directly (no fp16 SBUF staging): kills ~27M
    elements of copy traffic that v1 split between DVE and ScalarE.
  - software pipelining at unit granularity (unit = (head-pair, q-block)):
    the ctx matmuls + normalization of unit u-1, the K/Q projections of
    pair p+1, and the finished q-block's output projection all run as PE
    "fillers" inside unit u's score stream, so no engine ever sits behind
    a serial phase boundary.
  - normalization: reciprocal of the denominators row -> PE ones-matmul
    partition-broadcast -> multiply, all given >=2 emission slots of
    slack so the PE never blocks on the DVE chain.
  - PSUM: tag "sc" = 3 x [128,1024] f32 (scores ring; kq/V/out tiles
    sublease the same slots), tag "ctx" = 1 x [65,1024] f32 (ctx psum,
    also hosts the broadcast tile) -> exactly 8 banks.

Matmul operands are bf16 (full PE rate; ~0.4% rel error, inside the 2e-2
gate); PSUM accumulation fp32.
"""

import numpy as np
from contextlib import ExitStack

import ml_dtypes

import concourse.bass as bass
import concourse.bacc as bacc
import concourse.tile as tile
from concourse import mybir
from concourse import bass_utils

F32 = mybir.dt.float32
F32R = mybir.dt.float32r
BF16 = mybir.dt.bfloat16
AF = mybir.ActivationFunctionType
OP = mybir.AluOpType

P = 128
D = 768
H = 12
DH = 64
NE = D // P          # 6 e-chunks (head pairs)
ND = D // P          # 6 d-chunks
SCALE = 0.125
S_FULL = 1500
QB = 512
EH = 384             # half of D for the V projection moving dim


def _chunks(total, size):
    out = []
    o = 0
    while o < total:
        out.append((o, min(size, total - o)))
        o += size
    return out


def build_attention(tc, ctx, xT, wqT, wkT, wvT, woT, bqs, out, S, reps=1):
    """Emit the single-core attention program.

    xT:  [D, S] bf16 DRAM     (x^T for this batch element)
    wqT/wkT/wvT/woT: [D, D] bf16 DRAM  (W.T of the torch-Linear weights)
    bqs: [P, NE] f32 DRAM     (0.125*bq laid out [partition, e-chunk])
    out: [S, D] f32 DRAM      (missing the constant bv@Wo.T+bo row)
    """
    nc = tc.nc
    SC = _chunks(S, P)            # k-chunks, e.g. 11x128 + 92
    QBS = _chunks(S, QB)          # q-blocks 512/512/476
    NSC = len(SC)
    NQ = len(QBS)

    const = ctx.enter_context(tc.tile_pool(name="const", bufs=1))
    qkv = ctx.enter_context(tc.tile_pool(name="qkv", bufs=1))
    ps = ctx.enter_context(tc.tile_pool(name="ps", bufs=3, space="PSUM"))
    cps_pool = ctx.enter_context(tc.tile_pool(name="cpsp", bufs=1, space="PSUM"))
    e_pool = ctx.enter_context(tc.tile_pool(name="epool", bufs=2))
    nrm_pool = ctx.enter_context(tc.tile_pool(name="nrm", bufs=2))
    ctxn_pool = ctx.enter_context(tc.tile_pool(name="ctxn", bufs=3))
    out_sb_pool = ctx.enter_context(tc.tile_pool(name="outsb", bufs=3))
    kt_pool = ctx.enter_context(tc.tile_pool(name="ktp", bufs=2))
    x_pool = ctx.enter_context(tc.tile_pool(name="xp", bufs=1))

    # ---- persistent operands, loaded ONCE (outside the rep loop) ----
    bq_sb = const.tile([P, NE], F32)
    nc.sync.dma_start(out=bq_sb[:], in_=bqs)
    w_sbs = {}
    for key, wdram in (("q", wqT), ("k", wkT), ("v", wvT), ("o", woT)):
        w_sb = const.tile([P, ND, D], BF16, name=f"w_{key}")
        for dc in range(ND):
            nc.sync.dma_start(out=w_sb[:, dc, :],
                              in_=wdram[dc * P:(dc + 1) * P, :])
        w_sbs[key] = w_sb
    ones_sb = const.tile([1, DH], F32)
    nc.vector.memset(ones_sb[:], 1.0)

    V = qkv.tile([P, NSC, H * (DH + 1)], BF16)   # per-head 65th ones column
    # Fill all of V with 1.0 once: the projection evictions overwrite the
    # 64 data columns per head, leaving column DH as the all-ones column
    # that accumulates softmax denominators in the ctx matmul. bf16 memset
    # isn't a valid ISA op, so memset the f32-bitcast view with the bit
    # pattern of two packed bf16 1.0s (0x3F803F80).
    two_bf16_ones = float(np.frombuffer(
        np.uint32(0x3F803F80).tobytes(), dtype=np.float32)[0])
    nc.vector.memset(V[:, :, :].bitcast(F32), two_bf16_ones)

    def body():
        _emit_body(tc, nc, xT, out, S, SC, QBS, NSC, NQ,
                   V, bq_sb, w_sbs, ones_sb, ps, cps_pool, e_pool,
                   nrm_pool, ctxn_pool, out_sb_pool, kt_pool, x_pool)

    if reps == 1:
        body()
    else:
        with tc.For_i(0, reps, 1):
            body()


def _emit_body(tc, nc, xT, out, S, SC, QBS, NSC, NQ,
               V, bq_sb, w_sbs, ones_sb, ps, cps_pool, e_pool,
               nrm_pool, ctxn_pool, out_sb_pool, kt_pool, x_pool):

    # x^T for this rep: 6 flat 2D DMAs (contiguous source rows lower to
    # single hardware descriptors; a fancy 3D pattern costs ~30x more).
    xT_sb = x_pool.tile([P, ND, S], BF16, tag="xt", name="xT_sb")
    for dc in range(ND):
        nc.sync.dma_start(out=xT_sb[:, dc, :],
                          in_=xT[dc * P:(dc + 1) * P, :])

    # per-q-block normalized ctx [e, s], persistent across pairs in a rep
    cns = [ctxn_pool.tile([P, NE, QB], BF16, tag="cn", name=f"cn{_q}")
           for _q in range(NQ)]

    # ---------------- per-piece emitters ----------------

    def emit_kq_tile(pr, kind, qi):
        """One K or Q projection psum tile (all 6 dc accumulations) plus
        its DVE eviction — complete within one emission slot."""
        (q0, qw) = QBS[qi]
        w_sb = w_sbs[kind]
        kq_ps = ps.tile([P, 1024], F32, tag="sc", name="kq_ps")
        for dc in range(ND):
            nc.tensor.matmul(
                kq_ps[:, :qw],
                w_sb[:, dc, pr * P:(pr + 1) * P],
                xT_sb[:, dc, q0:q0 + qw],
                start=(dc == 0), stop=(dc == ND - 1))
        kt_t, qt_t = kt_tiles[pr % 2]
        if kind == "q":
            nc.vector.tensor_scalar(
                out=qt_t[:, q0:q0 + qw], in0=kq_ps[:, :qw],
                scalar1=SCALE, scalar2=bq_sb[:, pr:pr + 1],
                op0=OP.mult, op1=OP.add)
        else:
            nc.vector.tensor_copy(out=kt_t[:, q0:q0 + qw],
                                  in_=kq_ps[:, :qw])

    def emit_v_chunk(kc, eh):
        """V projection for one (k-chunk, head-half): eh=0 covers heads
        0-5 (needed from pair 0's ctx on), eh=1 heads 6-11 (first needed
        by pair 3) — so the eh=1 half spreads as filler into pairs 0-2."""
        (s0, sw) = SC[kc]
        v_ps = ps.tile([P, 1024], F32, tag="sc", name="v_ps")
        for dc in range(ND):
            nc.tensor.matmul(
                v_ps[:sw, :EH],
                xT_sb[:, dc, s0:s0 + sw],
                w_sbs["v"][:, dc, eh * EH:(eh + 1) * EH],
                start=(dc == 0), stop=(dc == ND - 1))
        vh = V[:sw, kc, :].rearrange("p (h w) -> p h w", w=DH + 1)
        nc.vector.tensor_copy(
            out=vh[:, eh * (EH // DH):(eh + 1) * (EH // DH), 0:DH],
            in_=v_ps[:sw, :EH].rearrange("p (h w) -> p h w", w=DH))

    def emit_scores_kc(u, kc, e_all):
        """QK^T for one k-chunk into a fresh sc psum tile, then exp it
        straight out of PSUM into e_all (full-width: the ragged q-block's
        dead columns exp garbage nobody reads)."""
        (pr, qi) = u
        (q0, qw) = QBS[qi]
        (k0, kw) = SC[kc]
        kt_t, qt_t = kt_tiles[pr % 2]
        sp = ps.tile([P, 1024], F32, tag="sc", name="sp")
        for hi in range(2):
            nc.tensor.matmul(
                sp[:kw, hi * 512:hi * 512 + qw],
                kt_t[hi * DH:(hi + 1) * DH, k0:k0 + kw],
                qt_t[hi * DH:(hi + 1) * DH, q0:q0 + qw],
                start=True, stop=True)
        nc.scalar.activation(out=e_all[:kw, kc, :], in_=sp[:kw, :],
                             func=AF.Exp)

    def emit_ctx_kc(u, kc, e_all, cps):
        (pr, qi) = u
        (q0, qw) = QBS[qi]
        (k0, kw) = SC[kc]
        for hi in range(2):
            h = 2 * pr + hi
            nc.tensor.matmul(
                cps[:, hi * 512:hi * 512 + qw],
                V[:kw, kc, h * (DH + 1):(h + 1) * (DH + 1)],
                e_all[:kw, kc, hi * 512:hi * 512 + qw],
                start=(kc == 0), stop=(kc == NSC - 1))

    def emit_norm_a(u, cps, st):
        """craw copy + reciprocal: first half of normalization; releases
        the ctx psum slot for the next unit."""
        craw = nrm_pool.tile([DH, 1024], F32, tag="craw", name="craw")
        nc.vector.tensor_copy(out=craw[:], in_=cps[0:DH, :])
        rc = nrm_pool.tile([1, 1024], F32R, tag="rc", name="rc")
        with nc.allow_low_precision(reason="f32r reciprocal, fp32 bits"):
            nc.vector.reciprocal(out=rc[:], in_=cps[DH:DH + 1, :])
        st["craw"], st["rc"] = craw, rc

    def emit_norm_b(u, st):
        """ones-matmul partition-broadcast of the reciprocal row."""
        rb_ps = cps_pool.tile([DH, 1024], F32, tag="ctx", name="rb_ps")
        for hi in range(2):  # matmul output must stay within one psum bank
            co = hi * 512
            nc.tensor.matmul(rb_ps[:, co:co + 512],
                             ones_sb[:, :].bitcast(F32R),
                             st["rc"][:, co:co + 512], start=True, stop=True)
        st["rb"] = rb_ps

    def emit_norm_c(u, st):
        (pr, qi) = u
        (q0, qw) = QBS[qi]
        for hi in range(2):
            co = hi * 512
            nc.vector.tensor_tensor(
                out=cns[qi][hi * DH:(hi + 1) * DH, pr, 0:qw],
                in0=st["craw"][:, co:co + qw], in1=st["rb"][:, co:co + qw],
                op=OP.mult)

    out_tiles = {}

    def emit_out_tile(qi, sc_i, half):
        """One output-projection psum tile (6 ec accumulations) + eviction
        (+ DMA store when the second half of the s-chunk completes)."""
        (q0, qw) = QBS[qi]
        (s0, sw) = _chunks(qw, P)[sc_i]
        (o0, ow) = ((0, 512), (512, 256))[half]
        op_t = ps.tile([P, 1024], F32, tag="sc", name="op_t")
        for ec in range(NE):
            nc.tensor.matmul(
                op_t[:sw, :ow],
                cns[qi][:, ec, s0:s0 + sw],
                w_sbs["o"][:, ec, o0:o0 + ow],
                start=(ec == 0), stop=(ec == NE - 1))
        if half == 0:
            out_tiles[(qi, sc_i)] = out_sb_pool.tile(
                [P, D], F32, tag="ot", name=f"ot{qi}_{sc_i}")
        ot = out_tiles[(qi, sc_i)]
        nc.vector.tensor_copy(out=ot[:sw, o0:o0 + ow], in_=op_t[:sw, :ow])
        if half == 1:
            nc.sync.dma_start(out=out[q0 + s0:q0 + s0 + sw, :],
                              in_=ot[:sw, :])

    def phase3_items(qi):
        return [(qi, sc_i, half)
                for sc_i in range(len(_chunks(QBS[qi][1], P)))
                for half in range(2)]

    # ---------------- the software-pipelined schedule ----------------
    #
    # Units run in (pair-major) order; inside unit u's 12-slot score
    # stream we interleave, as PE fillers:
    #   - ctx matmuls of unit u-1 (slots 0-4), its normalization
    #     (craw+recip at 5, broadcast at 8, multiplies at 10),
    #   - K/Q projection tiles of pair p+1 (slots 2/5/8 of qi=1,2 units),
    #   - output-projection tiles of a completed q-block (slots 2..9 of
    #     the following unit),
    #   - the V projection (pair 0, q-block 0 only).
    # Everything left at the end (ctx+norm of the last unit, the last two
    # output projections) drains after the unit loop; under reps it
    # overlaps the next rep's x-load and pair-0 K/Q window.

    kt_tiles = [None, None]

    def alloc_kt(pr):
        kt_tiles[pr % 2] = (
            kt_pool.tile([P, S], BF16, tag="kt", name=f"kt{pr}"),
            kt_pool.tile([P, S], BF16, tag="qt", name=f"qt{pr}"))

    units = [(pr, qi) for pr in range(NE) for qi in range(NQ)]
    # ctx k-chunks of the pending unit, compressed into slots 0-4
    CTX_SLOT_BOUNDS = [0, 3, 6, 8, 10, NSC]

    # pair 0 K/Q window at rep start (nothing to hide it behind within
    # the rep; across reps it overlaps the previous rep's drain)
    alloc_kt(0)
    for qi in range(NQ):
        for kind in ("k", "q"):
            emit_kq_tile(0, kind, qi)

    pending = None       # (unit, e_all, cps, state) awaiting ctx+norm
    phase3_queue = []    # out-projection tiles ready to fill into slots

    # eh=1 V chunks as (unit -> {slot: kc}) spread over pairs 0-2 (first
    # consumer is pair 3's ctx, one full pair of slack remains)
    veh1_plan = {}
    _v_units = [(0, 1), (0, 2), (1, 0), (1, 1), (1, 2), (2, 0)]
    for _i in range(NSC):
        veh1_plan.setdefault(_v_units[_i // 2], {})[(6, 9)[_i % 2]] = _i

    for ui, u in enumerate(units):
        (pr, qi) = u
        e_all = e_pool.tile([P, NSC, 1024], BF16, tag="e", name=f"e{ui}")
        if pending is not None:
            pu, pe, pst = pending
            pcps = cps_pool.tile([DH + 1, 1024], F32, tag="ctx", name="cps")
        # kq filler plan for pair p+1: 2 tiles per unit across all three
        # q-blocks (pair 0: 3 tiles in qi 1 and 2 — its qi=0 unit carries
        # the eh=0 V projection instead)
        kq_plan = {}
        if pr + 1 < NE:
            seq = [("k", 0), ("k", 1), ("k", 2), ("q", 0), ("q", 1), ("q", 2)]
            if pr == 0:
                if qi == 1:
                    alloc_kt(1)
                    kq_plan = {2: seq[0], 5: seq[1], 8: seq[2]}
                elif qi == 2:
                    kq_plan = {2: seq[3], 5: seq[4], 8: seq[5]}
            else:
                if qi == 0:
                    alloc_kt(pr + 1)
                kq_plan = {3: seq[2 * qi], 7: seq[2 * qi + 1]}

        for kc in range(NSC):
            emit_scores_kc(u, kc, e_all)
            if pending is not None:
                if kc < 5:
                    for pkc in range(CTX_SLOT_BOUNDS[kc],
                                     CTX_SLOT_BOUNDS[kc + 1]):
                        emit_ctx_kc(pu, pkc, pe, pcps)
                elif kc == 5:
                    emit_norm_a(pu, pcps, pst)
                elif kc == 8:
                    emit_norm_b(pu, pst)
                elif kc == 10:
                    emit_norm_c(pu, pst)
                    if pu[0] == NE - 1:
                        phase3_queue.extend(phase3_items(pu[1]))
            if kc in kq_plan:
                kind, j = kq_plan[kc]
                emit_kq_tile(pr + 1, kind, j)
            if u == (0, 0):
                emit_v_chunk(kc, 0)
            kc_v = veh1_plan.get(u, {}).get(kc)
            if kc_v is not None:
                emit_v_chunk(kc_v, 1)
            if 2 <= kc <= 9 and phase3_queue:
                emit_out_tile(*phase3_queue.pop(0))

        pending = (u, e_all, {})

    # ---------------- drain ----------------
    pu, pe, pst = pending
    pcps = cps_pool.tile([DH + 1, 1024], F32, tag="ctx", name="cps_last")
    for pkc in range(NSC):
        emit_ctx_kc(pu, pkc, pe, pcps)
        if phase3_queue:                      # phase3 of q-block 1
            emit_out_tile(*phase3_queue.pop(0))
    emit_norm_a(pu, pcps, pst)
    emit_norm_b(pu, pst)
    emit_norm_c(pu, pst)
    phase3_queue.extend(phase3_items(pu[1]))
    while phase3_queue:
        emit_out_tile(*phase3_queue.pop(0))


def build_nc(S=S_FULL, reps=1):
    nc = bacc.Bacc("TRN2", target_bir_lowering=False, debug=False,
                   enable_asserts=False, num_devices=1)
    xT = nc.dram_tensor("xT", [D, S], BF16, kind="ExternalInput").ap()
    wqT = nc.dram_tensor("wqT", [D, D], BF16, kind="ExternalInput").ap()
    wkT = nc.dram_tensor("wkT", [D, D], BF16, kind="ExternalInput").ap()
    wvT = nc.dram_tensor("wvT", [D, D], BF16, kind="ExternalInput").ap()
    woT = nc.dram_tensor("woT", [D, D], BF16, kind="ExternalInput").ap()
    bqs = nc.dram_tensor("bqs", [P, NE], F32, kind="ExternalInput").ap()
    out = nc.dram_tensor("out", [S, D], F32, kind="ExternalOutput").ap()
    with tile.TileContext(nc) as tc:
        with ExitStack() as ctx:
            build_attention(tc, ctx, xT, wqT, wkT, wvT, woT, bqs, out, S, reps)
    nc.compile()
    return nc


_NC_CACHE = {}


def _get_nc(S=S_FULL, reps=1):
    if (S, reps) not in _NC_CACHE:
        _NC_CACHE[(S, reps)] = build_nc(S, reps)
    return _NC_CACHE[(S, reps)]


def prep_inputs(x, Wq, bq, Wk, Wv, bv, Wo, bo):
    x = np.asarray(x, dtype=np.float32)
    Wq = np.asarray(Wq, dtype=np.float32)
    Wk = np.asarray(Wk, dtype=np.float32)
    Wv = np.asarray(Wv, dtype=np.float32)
    Wo = np.asarray(Wo, dtype=np.float32)
    bq = np.asarray(bq, dtype=np.float32)
    bv = np.asarray(bv, dtype=np.float32)
    bo = np.asarray(bo, dtype=np.float32)
    bf = ml_dtypes.bfloat16
    xT = np.ascontiguousarray(x.transpose(0, 2, 1)).astype(bf)
    base = {
        "wqT": np.ascontiguousarray(Wq.T).astype(bf),
        "wkT": np.ascontiguousarray(Wk.T).astype(bf),
        "wvT": np.ascontiguousarray(Wv.T).astype(bf),
        "woT": np.ascontiguousarray(Wo.T).astype(bf),
        "bqs": np.ascontiguousarray((SCALE * bq).reshape(NE, P).T),
    }
    const_row = (bv @ Wo.T + bo).astype(np.float32)
    in_maps = [dict(base, xT=np.ascontiguousarray(xT[b])) for b in range(x.shape[0])]
    return in_maps, const_row


def kernel(x, Wq, bq, Wk, Wv, bv, Wo, bo):
    in_maps, const_row = prep_inputs(x, Wq, bq, Wk, Wv, bv, Wo, bo)
    nc = _get_nc(x.shape[1])
    res = bass_utils.run_bass_kernel_spmd(
        nc, in_maps, core_ids=list(range(len(in_maps))))
    out = np.stack([r["out"] for r in res.results])
    return (out + const_row[None, None, :]).astype(np.float32)
